# revision 10
# baseline (speedup 1.0000x reference)
"""Bidirectional attention kernel for Trainium2 (8 NeuronCores, SPMD).

Sharding: core = (batch b, feat-token quarter iq). Each core processes
NI = 8192 feat tokens for one batch across all 4 heads:
  - qv projection (PE), per-head scores in both orientations (PE),
  - exp via ScalarE (fused with PSUM evac),
  - feat-direction: Pfeat = MV2ext^T @ E^T  (rows 0-63: W_feat_out-projected
    head contribution, row 64: softmax row-sum),
  - map-direction: Uext = [v;1]^T @ E  (rows 0-63: U, row 64: denominator).
Host does the tiny map-side prep (map qv projection, MV2 precompute) and the
final normalization / head-sum / map output projection.
"""
import numpy as np

import concourse.bass as bass
import concourse.bacc as bacc
import concourse.mybir as mybir
import concourse.tile as tile
from concourse.bass import ts
from concourse.bass_utils import run_bass_kernel_spmd

F32 = mybir.dt.float32
BF16 = mybir.dt.bfloat16
EXP = mybir.ActivationFunctionType.Exp

B, C, H, D, M = 2, 64, 4, 64, 512
N = 32 * 32 * 32
SCALE = float(D) ** -0.5


def build_nc(NI=N // 4, stages="all"):
    IB = NI // 128            # 128-token i-blocks
    NG2 = NI // 512           # 512-token i-groups
    nc = bacc.Bacc("TRN2")

    f_d = nc.dram_tensor("f", [C, NI], BF16, kind="ExternalInput")
    w_d = nc.dram_tensor("wqvT", [C, 512], BF16, kind="ExternalInput")
    mq_d = nc.dram_tensor("mapq", [2, 128, M], BF16, kind="ExternalInput")
    mv2_d = nc.dram_tensor("mv2e", [16, 128, 65], BF16, kind="ExternalInput")
    pf_d = nc.dram_tensor("pf", [H, 65, NI], F32, kind="ExternalOutput")
    ue_d = nc.dram_tensor("ue", [H, 65, M], F32, kind="ExternalOutput")

    with tile.TileContext(nc) as tc:
        with (
            tc.tile_pool(name="const", bufs=1) as const_pool,
            tc.tile_pool(name="qp", bufs=1) as q_pool,
            tc.tile_pool(name="vep", bufs=2) as ve_pool,
            tc.tile_pool(name="enp", bufs=3) as en_pool,
            tc.tile_pool(name="etp", bufs=3) as et_pool,
            tc.tile_pool(name="outp", bufs=3) as out_pool,
            tc.tile_pool(name="ps_sc", bufs=3, space="PSUM") as sc_psum,
            tc.tile_pool(name="ps_acc", bufs=1, space="PSUM") as acc_psum,
        ):
            f_sb = const_pool.tile([C, NI], BF16)
            w_sb = const_pool.tile([C, 512], BF16)
            mq01 = const_pool.tile([128, M], BF16)
            mq23 = const_pool.tile([128, M], BF16)
            mv2_sb = const_pool.tile([128, 16, 65], BF16)
            q01 = q_pool.tile([128, NI], BF16)
            q23 = q_pool.tile([128, NI], BF16)

            nc.sync.dma_start(f_sb[:, :], f_d[:, :])
            nc.sync.dma_start(w_sb[:, :], w_d[:, :])
            nc.sync.dma_start(mq01[:, :], mq_d[0])
            nc.sync.dma_start(mq23[:, :], mq_d[1])
            for t in range(16):
                nc.sync.dma_start(mv2_sb[:, t, :], mv2_d[t])

            # qv projection: q01 = [q_h0; q_h1], q23 = [q_h2; q_h3]
            for blk in range(2):
                qdst = q01 if blk == 0 else q23
                for c in range(NI // 512):
                    qv = sc_psum.tile([128, 2, 512], F32, tag="sc")
                    nc.tensor.matmul(qv[:, 0, :], w_sb[:, ts(blk, 128)],
                                     f_sb[:, ts(c, 512)], start=True, stop=True)
                    nc.vector.tensor_copy(qdst[:, ts(c, 512)], qv[:, 0, :])

            for h in (range(H) if stages != "qv" else []):
                off = (h % 2) * 64
                qsb = q01 if h < 2 else q23
                mqsb = mq01 if h < 2 else mq23

                # ---- prestage v^T (with ones column) for this head ----
                ve = ve_pool.tile([128, IB, 65], BF16, tag="ve")
                nc.vector.memset(ve[:, :, 64], 1.0)
                for ib in range(IB):
                    vt = sc_psum.tile([128, 2, 512], F32, tag="sc")
                    nc.tensor.matmul(vt[:, 0, 0:64], f_sb[:, ts(ib, 128)],
                                     w_sb[:, 256 + h * 64: 320 + h * 64],
                                     start=True, stop=True)
                    nc.vector.tensor_copy(ve[:, ib, 0:64], vt[:, 0, 0:64])

                if stages == "vt":
                    continue
                # ---- fused pass: scores -> exp -> {U accum, xbar transpose
                #      -> Pfeat} ----
                u = acc_psum.tile([65, M], F32, tag="u")
                for g in range(NG2):
                    et = et_pool.tile([128, 4, 512], BF16, tag="et")
                    for half in range(2):
                        s1 = sc_psum.tile([128, 2, 512], F32, tag="sc")
                        for k in range(2):
                            ib = g * 4 + half * 2 + k
                            nc.tensor.matmul(s1[:, k, :],
                                             qsb[off:off + 64, ts(ib, 128)],
                                             mqsb[off:off + 64, :],
                                             start=True, stop=True)
                        en = en_pool.tile([128, 2, 512], BF16, tag="en")
                        nc.scalar.activation(en[:, :, :], s1[:, :, :], EXP,
                                             scale=SCALE)
                        for k in range(2):
                            ib = g * 4 + half * 2 + k
                            pos = half * 2 + k
                            nc.tensor.matmul(u[:, :], ve[:, ib, :], en[:, k, :],
                                             start=(ib == 0), stop=(ib == IB - 1))
                            for jb in range(4):
                                nc.sync.dma_start_transpose(
                                    et[:, jb, ts(pos, 128)],
                                    en[:, k, ts(jb, 128)])
                    pf = acc_psum.tile([65, 512], F32, tag="pf")
                    for jb in range(4):
                        nc.tensor.matmul(pf[:, :], mv2_sb[:, h * 4 + jb, :],
                                         et[:, jb, :],
                                         start=(jb == 0), stop=(jb == 3))
                    pf_sb = out_pool.tile([65, 512], F32, tag="pf_sb")
                    nc.vector.tensor_copy(pf_sb[:, :], pf[:, :])
                    nc.sync.dma_start(pf_d[h, :, ts(g, 512)], pf_sb[:, :])
                u_sb = out_pool.tile([65, M], F32, tag="u_sb")
                nc.vector.tensor_copy(u_sb[:, :], u[:, :])
                nc.sync.dma_start(ue_d[h], u_sb[:, :])

    nc.compile()
    return nc


_PERM = np.array([d * 4 + h for h in range(H) for d in range(D)])


def host_prep(feat, semantic_map, w_feat_qv, w_map_qv, w_feat_out, NI):
    f = feat.reshape(B, C, N)
    m = semantic_map.reshape(B, 128, M)
    wq = w_feat_qv[:256][_PERM]
    wv = w_feat_qv[256:][_PERM]
    wqvT = np.ascontiguousarray(np.concatenate([wq, wv], 0).T)

    mqv = np.einsum('oc,bcm->bom', w_map_qv, m)
    map_q = mqv[:, :256][:, _PERM].reshape(B, H, D, M)
    map_v = mqv[:, 256:][:, _PERM].reshape(B, H, D, M)
    wfo_h = w_feat_out[:, _PERM].reshape(64, H, D).transpose(1, 0, 2)  # (H,64,D)

    mapq_dev = np.ascontiguousarray(map_q.reshape(B, 2, 2, D, M)
                                    .reshape(B, 2, 128, M))
    mv2e = np.empty((B, H, M, 65), np.float32)
    for b in range(B):
        for h in range(H):
            mv2e[b, h, :, :64] = (wfo_h[h] @ map_v[b, h]).T
            mv2e[b, h, :, 64] = 1.0
    mv2_dev = np.ascontiguousarray(mv2e.reshape(B, H, 4, 128, 65)
                                   .reshape(B, 16, 128, 65))

    import ml_dtypes
    bf16 = ml_dtypes.bfloat16
    in_maps = []
    for core in range(8):
        b, iq = core // 4, core % 4
        in_maps.append({
            "f": np.ascontiguousarray(f[b, :, iq * NI:(iq + 1) * NI]).astype(bf16),
            "wqvT": wqvT.astype(bf16),
            "mapq": mapq_dev[b].astype(bf16),
            "mv2e": mv2_dev[b].astype(bf16),
        })
    return in_maps


def host_post(results, w_map_out, NI):
    wmo_h = w_map_out[:, _PERM].reshape(128, H, D).transpose(1, 0, 2)  # (H,128,D)
    feat_out = np.empty((B, 64, N), np.float32)
    U = np.zeros((B, H, D, M), np.float32)
    den = np.zeros((B, H, M), np.float32)
    for core in range(8):
        b, iq = core // 4, core % 4
        pf = results[core]["pf"]
        feat_out[b, :, iq * NI:(iq + 1) * NI] = \
            (pf[:, :64, :] / pf[:, 64:65, :]).sum(0)
        ue = results[core]["ue"]
        U[b] += ue[:, :64, :]
        den[b] += ue[:, 64, :]
    map_out = np.zeros((B, 128, M), np.float32)
    for b in range(B):
        for h in range(H):
            map_out[b] += wmo_h[h] @ (U[b, h] / den[b, h][None, :])
    return (feat_out.reshape(B, 64, 32, 32, 32),
            map_out.reshape(B, 128, 8, 8, 8))


_NC_CACHE = {}


def _get_nc(NI):
    if NI not in _NC_CACHE:
        _NC_CACHE[NI] = build_nc(NI)
    return _NC_CACHE[NI]


def kernel(feat, semantic_map, w_feat_qv, w_map_qv, w_feat_out, w_map_out,
           _trace=False):
    NI = N // 4
    feat = np.asarray(feat, np.float32)
    semantic_map = np.asarray(semantic_map, np.float32)
    w_feat_qv = np.asarray(w_feat_qv, np.float32)
    w_map_qv = np.asarray(w_map_qv, np.float32)
    w_feat_out = np.asarray(w_feat_out, np.float32)
    w_map_out = np.asarray(w_map_out, np.float32)

    nc = _get_nc(NI)
    in_maps = host_prep(feat, semantic_map, w_feat_qv, w_map_qv, w_feat_out, NI)
    res = run_bass_kernel_spmd(nc, in_maps, list(range(8)), trace=_trace)
    out = host_post(res.results, w_map_out, NI)
    if _trace:
        return out, res
    return out


# revision 11
# speedup vs baseline: 2.8207x; 2.8207x over previous
"""Bidirectional attention kernel for Trainium2 (8 NeuronCores, SPMD).

Sharding: core = (batch b, feat-token quarter iq). Each core processes
NI = 8192 feat tokens for one batch across all 4 heads:
  - qv projection (PE), per-head scores in both orientations (PE),
  - exp via ScalarE (fused with PSUM evac),
  - feat-direction: Pfeat = MV2ext^T @ E^T  (rows 0-63: W_feat_out-projected
    head contribution, row 64: softmax row-sum),
  - map-direction: Uext = [v;1]^T @ E  (rows 0-63: U, row 64: denominator).
Host does the tiny map-side prep (map qv projection, MV2 precompute) and the
final normalization / head-sum / map output projection.
"""
import numpy as np

import concourse.bass as bass
import concourse.bacc as bacc
import concourse.mybir as mybir
import concourse.tile as tile
from concourse.bass import ts
from concourse.bass_utils import run_bass_kernel_spmd

F32 = mybir.dt.float32
BF16 = mybir.dt.bfloat16
EXP = mybir.ActivationFunctionType.Exp

B, C, H, D, M = 2, 64, 4, 64, 512
N = 32 * 32 * 32
SCALE = float(D) ** -0.5


def build_nc(NI=N // 4, stages="all"):
    IB = NI // 128            # 128-token i-blocks
    NG2 = NI // 512           # 512-token i-groups
    nc = bacc.Bacc("TRN2")

    f_d = nc.dram_tensor("f", [C, NI], BF16, kind="ExternalInput")
    w_d = nc.dram_tensor("wqvT", [C, 512], BF16, kind="ExternalInput")
    mq_d = nc.dram_tensor("mapq", [2, 128, M], BF16, kind="ExternalInput")
    mv2_d = nc.dram_tensor("mv2e", [16, 128, 65], BF16, kind="ExternalInput")
    pf_d = nc.dram_tensor("pf", [H, 65, NI], F32, kind="ExternalOutput")
    ue_d = nc.dram_tensor("ue", [H, 65, M], F32, kind="ExternalOutput")

    with tile.TileContext(nc) as tc:
        with (
            tc.tile_pool(name="const", bufs=1) as const_pool,
            tc.tile_pool(name="qp", bufs=1) as q_pool,
            tc.tile_pool(name="vep", bufs=2) as ve_pool,
            tc.tile_pool(name="enp", bufs=3) as en_pool,
            tc.tile_pool(name="etp", bufs=3) as et_pool,
            tc.tile_pool(name="outp", bufs=3) as out_pool,
            tc.tile_pool(name="ps_sc", bufs=3, space="PSUM") as sc_psum,
            tc.tile_pool(name="ps_acc", bufs=1, space="PSUM") as acc_psum,
        ):
            f_sb = const_pool.tile([C, NI], BF16)
            w_sb = const_pool.tile([C, 512], BF16)
            mq01 = const_pool.tile([128, M], BF16)
            mq23 = const_pool.tile([128, M], BF16)
            mv2_sb = const_pool.tile([128, 16, 65], BF16)
            q01 = q_pool.tile([128, NI], BF16)
            q23 = q_pool.tile([128, NI], BF16)

            nc.sync.dma_start(f_sb[:, :], f_d[:, :])
            nc.sync.dma_start(w_sb[:, :], w_d[:, :])
            nc.sync.dma_start(mq01[:, :], mq_d[0])
            nc.sync.dma_start(mq23[:, :], mq_d[1])
            for t in range(16):
                nc.sync.dma_start(mv2_sb[:, t, :], mv2_d[t])

            # qv projection: q01 = [q_h0; q_h1], q23 = [q_h2; q_h3]
            for blk in range(2):
                qdst = q01 if blk == 0 else q23
                for c in range(NI // 512):
                    qv = sc_psum.tile([128, 2, 512], F32, tag="sc")
                    nc.tensor.matmul(qv[:, 0, :], w_sb[:, ts(blk, 128)],
                                     f_sb[:, ts(c, 512)], start=True, stop=True)
                    nc.vector.tensor_copy(qdst[:, ts(c, 512)], qv[:, 0, :])

            for h in (range(H) if stages != "qv" else []):
                off = (h % 2) * 64
                qsb = q01 if h < 2 else q23
                mqsb = mq01 if h < 2 else mq23

                # ---- prestage v^T (with ones column) for this head ----
                ve = ve_pool.tile([128, IB, 65], BF16, tag="ve")
                nc.vector.memset(ve[:, :, 64], 1.0)
                for ib in range(IB):
                    vt = sc_psum.tile([128, 2, 512], F32, tag="sc")
                    nc.tensor.matmul(vt[:, 0, 0:64], f_sb[:, ts(ib, 128)],
                                     w_sb[:, 256 + h * 64: 320 + h * 64],
                                     start=True, stop=True)
                    nc.vector.tensor_copy(ve[:, ib, 0:64], vt[:, 0, 0:64])

                if stages == "vt":
                    continue
                # ---- pass 1: natural scores -> exp -> U accumulation.
                # Emission software-pipelined: the u-matmuls for group g are
                # emitted AFTER the score-matmuls of group g+1 so the PE
                # instruction order never stalls on the exp of group g.
                u = acc_psum.tile([65, M], F32, tag="u")
                pend = None
                for g in range(IB // 2 + 1):
                    if g < IB // 2:
                        s1 = sc_psum.tile([128, 2, 512], F32, tag="sc")
                        for k in range(2):
                            ib = g * 2 + k
                            nc.tensor.matmul(s1[:, k, :],
                                             qsb[off:off + 64, ts(ib, 128)],
                                             mqsb[off:off + 64, :],
                                             start=True, stop=True)
                        en = en_pool.tile([128, 2, 512], BF16, tag="en")
                        nc.scalar.activation(en[:, :, :], s1[:, :, :], EXP,
                                             scale=SCALE)
                    if pend is not None:
                        en_p, g_p = pend
                        for k in range(2):
                            ib = g_p * 2 + k
                            nc.tensor.matmul(u[:, :], ve[:, ib, :],
                                             en_p[:, k, :],
                                             start=(ib == 0),
                                             stop=(ib == IB - 1))
                    if g < IB // 2:
                        pend = (en, g)
                u_sb = out_pool.tile([65, M], F32, tag="u_sb")
                nc.vector.tensor_copy(u_sb[:, :], u[:, :])
                nc.sync.dma_start(ue_d[h], u_sb[:, :])

                # ---- pass 2: transposed scores -> exp -> Pfeat, same
                # one-step-late emission of the consumer matmuls. ----
                pend = None
                pf_cur = None
                pf_g = None
                for step in range(2 * NG2 + 1):
                    if step < 2 * NG2:
                        g, half = step // 2, step % 2
                        st = sc_psum.tile([128, 2, 512], F32, tag="sc")
                        for q_ in range(2):
                            jb = half * 2 + q_
                            nc.tensor.matmul(st[:, q_, :],
                                             mqsb[off:off + 64, ts(jb, 128)],
                                             qsb[off:off + 64, ts(g, 512)],
                                             start=True, stop=True)
                        et = et_pool.tile([128, 2, 512], BF16, tag="et")
                        nc.scalar.activation(et[:, :, :], st[:, :, :], EXP,
                                             scale=SCALE)
                    if pend is not None:
                        et_p, g_p, half_p = pend
                        if half_p == 0:
                            pf_cur = acc_psum.tile([65, 512], F32, tag="pf")
                            pf_g = g_p
                        for q_ in range(2):
                            jb = half_p * 2 + q_
                            nc.tensor.matmul(pf_cur[:, :],
                                             mv2_sb[:, h * 4 + jb, :],
                                             et_p[:, q_, :],
                                             start=(jb == 0), stop=(jb == 3))
                        if half_p == 1:
                            pf_sb = out_pool.tile([65, 512], F32, tag="pf_sb")
                            nc.vector.tensor_copy(pf_sb[:, :], pf_cur[:, :])
                            nc.sync.dma_start(pf_d[h, :, ts(pf_g, 512)],
                                              pf_sb[:, :])
                    if step < 2 * NG2:
                        pend = (et, g, half)

    nc.compile()
    return nc


_PERM = np.array([d * 4 + h for h in range(H) for d in range(D)])


def host_prep(feat, semantic_map, w_feat_qv, w_map_qv, w_feat_out, NI):
    f = feat.reshape(B, C, N)
    m = semantic_map.reshape(B, 128, M)
    wq = w_feat_qv[:256][_PERM]
    wv = w_feat_qv[256:][_PERM]
    wqvT = np.ascontiguousarray(np.concatenate([wq, wv], 0).T)

    mqv = np.einsum('oc,bcm->bom', w_map_qv, m)
    map_q = mqv[:, :256][:, _PERM].reshape(B, H, D, M)
    map_v = mqv[:, 256:][:, _PERM].reshape(B, H, D, M)
    wfo_h = w_feat_out[:, _PERM].reshape(64, H, D).transpose(1, 0, 2)  # (H,64,D)

    mapq_dev = np.ascontiguousarray(map_q.reshape(B, 2, 2, D, M)
                                    .reshape(B, 2, 128, M))
    mv2e = np.empty((B, H, M, 65), np.float32)
    for b in range(B):
        for h in range(H):
            mv2e[b, h, :, :64] = (wfo_h[h] @ map_v[b, h]).T
            mv2e[b, h, :, 64] = 1.0
    mv2_dev = np.ascontiguousarray(mv2e.reshape(B, H, 4, 128, 65)
                                   .reshape(B, 16, 128, 65))

    import ml_dtypes
    bf16 = ml_dtypes.bfloat16
    in_maps = []
    for core in range(8):
        b, iq = core // 4, core % 4
        in_maps.append({
            "f": np.ascontiguousarray(f[b, :, iq * NI:(iq + 1) * NI]).astype(bf16),
            "wqvT": wqvT.astype(bf16),
            "mapq": mapq_dev[b].astype(bf16),
            "mv2e": mv2_dev[b].astype(bf16),
        })
    return in_maps


def host_post(results, w_map_out, NI):
    wmo_h = w_map_out[:, _PERM].reshape(128, H, D).transpose(1, 0, 2)  # (H,128,D)
    feat_out = np.empty((B, 64, N), np.float32)
    U = np.zeros((B, H, D, M), np.float32)
    den = np.zeros((B, H, M), np.float32)
    for core in range(8):
        b, iq = core // 4, core % 4
        pf = results[core]["pf"]
        feat_out[b, :, iq * NI:(iq + 1) * NI] = \
            (pf[:, :64, :] / pf[:, 64:65, :]).sum(0)
        ue = results[core]["ue"]
        U[b] += ue[:, :64, :]
        den[b] += ue[:, 64, :]
    map_out = np.zeros((B, 128, M), np.float32)
    for b in range(B):
        for h in range(H):
            map_out[b] += wmo_h[h] @ (U[b, h] / den[b, h][None, :])
    return (feat_out.reshape(B, 64, 32, 32, 32),
            map_out.reshape(B, 128, 8, 8, 8))


_NC_CACHE = {}


def _get_nc(NI):
    if NI not in _NC_CACHE:
        _NC_CACHE[NI] = build_nc(NI)
    return _NC_CACHE[NI]


def kernel(feat, semantic_map, w_feat_qv, w_map_qv, w_feat_out, w_map_out,
           _trace=False):
    NI = N // 4
    feat = np.asarray(feat, np.float32)
    semantic_map = np.asarray(semantic_map, np.float32)
    w_feat_qv = np.asarray(w_feat_qv, np.float32)
    w_map_qv = np.asarray(w_map_qv, np.float32)
    w_feat_out = np.asarray(w_feat_out, np.float32)
    w_map_out = np.asarray(w_map_out, np.float32)

    nc = _get_nc(NI)
    in_maps = host_prep(feat, semantic_map, w_feat_qv, w_map_qv, w_feat_out, NI)
    res = run_bass_kernel_spmd(nc, in_maps, list(range(8)), trace=_trace)
    out = host_post(res.results, w_map_out, NI)
    if _trace:
        return out, res
    return out


# revision 13
# speedup vs baseline: 3.8897x; 1.3790x over previous
"""Bidirectional attention kernel for Trainium2 (8 NeuronCores, SPMD).

Sharding: core = (batch b, feat-token quarter iq). Each core processes
NI = 8192 feat tokens for one batch across all 4 heads.

Heads are processed in pairs (even head on SBUF partitions 0-63, odd head
on 64-127):
  - score matmuls (K=64) run as concurrent 64x128 row tiles T0/T8,
  - exp on ScalarE straight out of PSUM (bf16 out),
  - U (map-direction) and Pfeat (feat-direction) matmuls (K=128) run as
    concurrent 128x64 column tiles (even head -> PSUM 0-63, odd -> 64-127),
  - softmax row-sums (over map tokens) are free-dim reduces on E-natural,
    denominators (over feat tokens) are free-dim reduces on E-transposed,
    both on VectorE.
Host does the tiny map-side prep (map qv projection, MV2 = W_feat_out @
map_v precompute) and the final normalization / head-sum / map projection.
"""
import numpy as np

import concourse.bass as bass
import concourse.bacc as bacc
import concourse.mybir as mybir
import concourse.tile as tile
from concourse.bass import ts
from concourse.bass_utils import run_bass_kernel_spmd

F32 = mybir.dt.float32
BF16 = mybir.dt.bfloat16
EXP = mybir.ActivationFunctionType.Exp
AXX = mybir.AxisListType.X
ADD = mybir.AluOpType.add

B, C, H, D, M = 2, 64, 4, 64, 512
N = 32 * 32 * 32
SCALE = float(D) ** -0.5


def build_nc(NI=N // 4):
    IB = NI // 128            # 128-token i-blocks
    NG2 = NI // 512           # 512-token i-groups
    nc = bacc.Bacc("TRN2")

    f_d = nc.dram_tensor("f", [C, NI], BF16, kind="ExternalInput")
    w_d = nc.dram_tensor("wqvT", [C, 512], BF16, kind="ExternalInput")
    mq_d = nc.dram_tensor("mapq", [2, 128, M], BF16, kind="ExternalInput")
    mv2_d = nc.dram_tensor("mv2e", [16, 128, 64], BF16, kind="ExternalInput")
    pf_d = nc.dram_tensor("pfp", [2, 128, NI], F32, kind="ExternalOutput")
    up_d = nc.dram_tensor("up", [2, 128, M], F32, kind="ExternalOutput")
    rs_d = nc.dram_tensor("rs", [H, 128, IB], F32, kind="ExternalOutput")
    den_d = nc.dram_tensor("den", [H, 128, 4 * NG2], F32, kind="ExternalOutput")

    with tile.TileContext(nc) as tc:
        with (
            tc.tile_pool(name="const", bufs=1) as const_pool,
            tc.tile_pool(name="qp", bufs=1) as q_pool,
            tc.tile_pool(name="vep", bufs=2) as ve_pool,
            tc.tile_pool(name="enp", bufs=4) as en_pool,
            tc.tile_pool(name="etp", bufs=4) as et_pool,
            tc.tile_pool(name="sump", bufs=2) as sum_pool,
            tc.tile_pool(name="outp", bufs=3) as out_pool,
            tc.tile_pool(name="ps_sc", bufs=3, space="PSUM") as sc_psum,
            tc.tile_pool(name="ps_acc", bufs=1, space="PSUM") as acc_psum,
        ):
            f_full = const_pool.tile([128, NI], BF16)
            w_full = const_pool.tile([128, 512], BF16)
            mq01 = const_pool.tile([128, M], BF16)
            mq23 = const_pool.tile([128, M], BF16)
            mv2_sb = const_pool.tile([128, 16, 64], BF16)
            q01 = q_pool.tile([128, NI], BF16)
            q23 = q_pool.tile([128, NI], BF16)

            nc.sync.dma_start(f_full[0:64, :], f_d[:, :])
            nc.sync.dma_start(f_full[64:128, :], f_d[:, :])
            nc.sync.dma_start(w_full[0:64, :], w_d[:, :])
            nc.sync.dma_start(w_full[64:128, :], w_d[:, :])
            nc.sync.dma_start(mq01[:, :], mq_d[0])
            nc.sync.dma_start(mq23[:, :], mq_d[1])
            for t in range(16):
                nc.sync.dma_start(mv2_sb[:, t, :], mv2_d[t])

            # qv projection, row-tiled pair: q01 = [q_h0; q_h1] from T0,
            # q23 = [q_h2; q_h3] from T8.
            for c in range(NI // 512):
                s = sc_psum.tile([128, 2, 512], F32, tag="sc")
                nc.tensor.matmul(s[:, 0, :], w_full[0:64, 0:128],
                                 f_full[0:64, ts(c, 512)],
                                 start=True, stop=True, tile_position=(0, 0))
                nc.tensor.matmul(s[:, 1, :], w_full[64:128, 128:256],
                                 f_full[64:128, ts(c, 512)],
                                 start=True, stop=True, tile_position=(64, 0))
                nc.vector.tensor_copy(q01[:, ts(c, 512)], s[:, 0, :])
                nc.vector.tensor_copy(q23[:, ts(c, 512)], s[:, 1, :])

            for p in range(2):
                he, ho = 2 * p, 2 * p + 1
                qsb = q01 if p == 0 else q23
                mqsb = mq01 if p == 0 else mq23

                # ---- prestage v^T for both heads of the pair ----
                ve_e = ve_pool.tile([128, IB, 64], BF16, tag="ve_e")
                ve_o = ve_pool.tile([128, IB, 64], BF16, tag="ve_o")
                for ib in range(IB):
                    vt = sc_psum.tile([128, 2, 512], F32, tag="sc")
                    nc.tensor.matmul(vt[:, 0, 0:64], f_full[0:64, ts(ib, 128)],
                                     w_full[0:64, 256 + he * 64: 320 + he * 64],
                                     start=True, stop=True,
                                     tile_position=(0, 0))
                    nc.tensor.matmul(vt[:, 1, 0:64],
                                     f_full[64:128, ts(ib, 128)],
                                     w_full[64:128, 256 + ho * 64: 320 + ho * 64],
                                     start=True, stop=True,
                                     tile_position=(64, 0))
                    nc.vector.tensor_copy(ve_e[:, ib, :], vt[:, 0, 0:64])
                    nc.vector.tensor_copy(ve_o[:, ib, :], vt[:, 1, 0:64])

                # ---- pass 1: natural scores -> exp -> U + rowsums ----
                u_pair = acc_psum.tile([128, M], F32, tag="u")
                rs_e = sum_pool.tile([128, IB], F32, tag="rs_e")
                rs_o = sum_pool.tile([128, IB], F32, tag="rs_o")
                for g in range(IB):
                    s = sc_psum.tile([128, 2, 512], F32, tag="sc")
                    nc.tensor.matmul(s[:, 0, :], qsb[0:64, ts(g, 128)],
                                     mqsb[0:64, :], start=True, stop=True,
                                     tile_position=(0, 0))
                    nc.tensor.matmul(s[:, 1, :], qsb[64:128, ts(g, 128)],
                                     mqsb[64:128, :], start=True, stop=True,
                                     tile_position=(64, 0))
                    en = en_pool.tile([128, 2, 512], BF16, tag="en")
                    nc.scalar.activation(en[:, :, :], s[:, :, :], EXP,
                                         scale=SCALE)
                    nc.vector.tensor_reduce(rs_e[:, g:g + 1], en[:, 0, :],
                                            axis=AXX, op=ADD)
                    nc.vector.tensor_reduce(rs_o[:, g:g + 1], en[:, 1, :],
                                            axis=AXX, op=ADD)
                    nc.tensor.matmul(u_pair[0:64, :], ve_e[:, g, :],
                                     en[:, 0, :], start=(g == 0),
                                     stop=(g == IB - 1), tile_position=(0, 0),
                                     skip_group_check=True)
                    nc.tensor.matmul(u_pair[64:128, :], ve_o[:, g, :],
                                     en[:, 1, :], start=(g == 0),
                                     stop=(g == IB - 1), tile_position=(0, 64),
                                     skip_group_check=True)
                u_sb = out_pool.tile([128, M], F32, tag="u_sb")
                nc.vector.tensor_copy(u_sb[:, :], u_pair[:, :])
                nc.sync.dma_start(up_d[p], u_sb[:, :])
                nc.sync.dma_start(rs_d[he], rs_e[:, :])
                nc.sync.dma_start(rs_d[ho], rs_o[:, :])

                # ---- pass 2: transposed scores -> exp -> Pfeat + dens ----
                den_e = sum_pool.tile([128, 4 * NG2], F32, tag="den_e")
                den_o = sum_pool.tile([128, 4 * NG2], F32, tag="den_o")
                for g in range(NG2):
                    pf_pair = acc_psum.tile([128, 512], F32, tag="pf")
                    for jb in range(4):
                        st = sc_psum.tile([128, 2, 512], F32, tag="sc")
                        nc.tensor.matmul(st[:, 0, :], mqsb[0:64, ts(jb, 128)],
                                         qsb[0:64, ts(g, 512)],
                                         start=True, stop=True,
                                         tile_position=(0, 0))
                        nc.tensor.matmul(st[:, 1, :],
                                         mqsb[64:128, ts(jb, 128)],
                                         qsb[64:128, ts(g, 512)],
                                         start=True, stop=True,
                                         tile_position=(64, 0))
                        et = et_pool.tile([128, 2, 512], BF16, tag="et")
                        nc.scalar.activation(et[:, :, :], st[:, :, :], EXP,
                                             scale=SCALE)
                        col = jb * NG2 + g
                        nc.vector.tensor_reduce(den_e[:, col:col + 1],
                                                et[:, 0, :], axis=AXX, op=ADD)
                        nc.vector.tensor_reduce(den_o[:, col:col + 1],
                                                et[:, 1, :], axis=AXX, op=ADD)
                        nc.tensor.matmul(pf_pair[0:64, :],
                                         mv2_sb[:, he * 4 + jb, :],
                                         et[:, 0, :], start=(jb == 0),
                                         stop=(jb == 3), tile_position=(0, 0),
                                         skip_group_check=True)
                        nc.tensor.matmul(pf_pair[64:128, :],
                                         mv2_sb[:, ho * 4 + jb, :],
                                         et[:, 1, :], start=(jb == 0),
                                         stop=(jb == 3), tile_position=(0, 64),
                                         skip_group_check=True)
                    pf_sb = out_pool.tile([128, 512], F32, tag="pf_sb")
                    nc.vector.tensor_copy(pf_sb[:, :], pf_pair[:, :])
                    nc.sync.dma_start(pf_d[p, :, ts(g, 512)], pf_sb[:, :])
                nc.sync.dma_start(den_d[he], den_e[:, :])
                nc.sync.dma_start(den_d[ho], den_o[:, :])

    nc.compile()
    return nc


_PERM = np.array([d * 4 + h for h in range(H) for d in range(D)])


def host_prep(feat, semantic_map, w_feat_qv, w_map_qv, w_feat_out, NI):
    import ml_dtypes
    bf16 = ml_dtypes.bfloat16
    f = feat.reshape(B, C, N)
    m = semantic_map.reshape(B, 128, M)
    wq = w_feat_qv[:256][_PERM]
    wv = w_feat_qv[256:][_PERM]
    wqvT = np.ascontiguousarray(np.concatenate([wq, wv], 0).T)

    mqv = np.einsum('oc,bcm->bom', w_map_qv, m)
    map_q = mqv[:, :256][:, _PERM].reshape(B, H, D, M)
    map_v = mqv[:, 256:][:, _PERM].reshape(B, H, D, M)
    wfo_h = w_feat_out[:, _PERM].reshape(64, H, D).transpose(1, 0, 2)  # (H,64,D)

    mapq_dev = np.ascontiguousarray(map_q.reshape(B, 2, 2, D, M)
                                    .reshape(B, 2, 128, M))
    mv2 = np.empty((B, H, M, 64), np.float32)
    for b in range(B):
        for h in range(H):
            mv2[b, h] = (wfo_h[h] @ map_v[b, h]).T
    mv2_dev = np.ascontiguousarray(mv2.reshape(B, H, 4, 128, 64)
                                   .reshape(B, 16, 128, 64))

    in_maps = []
    for core in range(8):
        b, iq = core // 4, core % 4
        in_maps.append({
            "f": np.ascontiguousarray(f[b, :, iq * NI:(iq + 1) * NI]).astype(bf16),
            "wqvT": wqvT.astype(bf16),
            "mapq": mapq_dev[b].astype(bf16),
            "mv2e": mv2_dev[b].astype(bf16),
        })
    return in_maps


def host_post(results, w_map_out, NI):
    IB = NI // 128
    wmo_h = w_map_out[:, _PERM].reshape(128, H, D).transpose(1, 0, 2)  # (H,128,D)
    feat_out = np.empty((B, 64, N), np.float32)
    U = np.zeros((B, H, D, M), np.float32)
    den = np.zeros((B, H, M), np.float32)
    for core in range(8):
        b, iq = core // 4, core % 4
        r = results[core]
        acc = np.zeros((64, NI), np.float32)
        for h in range(H):
            pfh = r["pfp"][h // 2, (h % 2) * 64:(h % 2) * 64 + 64, :]  # (64,NI)
            rsh = r["rs"][h].T.reshape(NI)                   # [ib,128] -> i
            acc += pfh / rsh[None, :]
            U[b, h] += r["up"][h // 2, (h % 2) * 64:(h % 2) * 64 + 64, :]
            den[b, h] += r["den"][h].reshape(128, 4, -1).sum(2).T.reshape(M)
        feat_out[b, :, iq * NI:(iq + 1) * NI] = acc
    map_out = np.zeros((B, 128, M), np.float32)
    for b in range(B):
        for h in range(H):
            map_out[b] += wmo_h[h] @ (U[b, h] / den[b, h][None, :])
    return (feat_out.reshape(B, 64, 32, 32, 32),
            map_out.reshape(B, 128, 8, 8, 8))


_NC_CACHE = {}


def _get_nc(NI):
    if NI not in _NC_CACHE:
        _NC_CACHE[NI] = build_nc(NI)
    return _NC_CACHE[NI]


def kernel(feat, semantic_map, w_feat_qv, w_map_qv, w_feat_out, w_map_out,
           _trace=False):
    NI = N // 4
    feat = np.asarray(feat, np.float32)
    semantic_map = np.asarray(semantic_map, np.float32)
    w_feat_qv = np.asarray(w_feat_qv, np.float32)
    w_map_qv = np.asarray(w_map_qv, np.float32)
    w_feat_out = np.asarray(w_feat_out, np.float32)
    w_map_out = np.asarray(w_map_out, np.float32)

    nc = _get_nc(NI)
    in_maps = host_prep(feat, semantic_map, w_feat_qv, w_map_qv, w_feat_out, NI)
    res = run_bass_kernel_spmd(nc, in_maps, list(range(8)), trace=_trace)
    out = host_post(res.results, w_map_out, NI)
    if _trace:
        return out, res
    return out


# revision 14
# speedup vs baseline: 4.2608x; 1.0954x over previous
"""Bidirectional attention kernel for Trainium2 (8 NeuronCores, SPMD).

Sharding: core = (batch b, feat-token quarter iq). Each core processes
NI = 8192 feat tokens for one batch across all 4 heads.

Heads are processed in pairs (even head on SBUF partitions 0-63, odd head
on 64-127):
  - score matmuls (K=64) run as concurrent 64x128 row tiles T0/T8,
  - exp on ScalarE straight out of PSUM (bf16 out),
  - U (map-direction) and Pfeat (feat-direction) matmuls (K=128) run as
    concurrent 128x64 column tiles (even head -> PSUM 0-63, odd -> 64-127),
  - softmax row-sums (over map tokens) are free-dim reduces on E-natural,
    denominators (over feat tokens) are free-dim reduces on E-transposed,
    both on VectorE.
Host does the tiny map-side prep (map qv projection, MV2 = W_feat_out @
map_v precompute) and the final normalization / head-sum / map projection.
"""
import numpy as np

import concourse.bass as bass
import concourse.bacc as bacc
import concourse.mybir as mybir
import concourse.tile as tile
from concourse.bass import ts
from concourse.bass_utils import run_bass_kernel_spmd

F32 = mybir.dt.float32
BF16 = mybir.dt.bfloat16
EXP = mybir.ActivationFunctionType.Exp
AXX = mybir.AxisListType.X
ADD = mybir.AluOpType.add

B, C, H, D, M = 2, 64, 4, 64, 512
N = 32 * 32 * 32
SCALE = float(D) ** -0.5


def build_nc(NI=N // 4):
    IB = NI // 128            # 128-token i-blocks
    NG2 = NI // 512           # 512-token i-groups
    nc = bacc.Bacc("TRN2")

    f_d = nc.dram_tensor("f", [C, NI], BF16, kind="ExternalInput")
    w_d = nc.dram_tensor("wqvT", [C, 512], BF16, kind="ExternalInput")
    mq_d = nc.dram_tensor("mapq", [2, 128, M], BF16, kind="ExternalInput")
    mv2_d = nc.dram_tensor("mv2e", [16, 128, 64], BF16, kind="ExternalInput")
    pf_d = nc.dram_tensor("pfp", [2, 128, NI], F32, kind="ExternalOutput")
    up_d = nc.dram_tensor("up", [2, 128, M], F32, kind="ExternalOutput")
    rs_d = nc.dram_tensor("rs", [H, 128, IB], F32, kind="ExternalOutput")
    den_d = nc.dram_tensor("den", [H, 128, 4 * NG2], F32, kind="ExternalOutput")

    with tile.TileContext(nc) as tc:
        with (
            tc.tile_pool(name="const", bufs=1) as const_pool,
            tc.tile_pool(name="qp", bufs=1) as q_pool,
            tc.tile_pool(name="vep", bufs=2) as ve_pool,
            tc.tile_pool(name="enp", bufs=4) as en_pool,
            tc.tile_pool(name="etp", bufs=4) as et_pool,
            tc.tile_pool(name="sump", bufs=2) as sum_pool,
            tc.tile_pool(name="outp", bufs=3) as out_pool,
            tc.tile_pool(name="ps_sc", bufs=3, space="PSUM") as sc_psum,
            tc.tile_pool(name="ps_acc", bufs=1, space="PSUM") as acc_psum,
        ):
            f_full = const_pool.tile([128, NI], BF16)
            w_full = const_pool.tile([128, 512], BF16)
            mq01 = const_pool.tile([128, M], BF16)
            mq23 = const_pool.tile([128, M], BF16)
            mv2_sb = const_pool.tile([128, 16, 64], BF16)
            q01 = q_pool.tile([128, NI], BF16)
            q23 = q_pool.tile([128, NI], BF16)

            nc.sync.dma_start(f_full[0:64, :], f_d[:, :])
            nc.sync.dma_start(f_full[64:128, :], f_d[:, :])
            nc.sync.dma_start(w_full[0:64, :], w_d[:, :])
            nc.sync.dma_start(w_full[64:128, :], w_d[:, :])
            nc.sync.dma_start(mq01[:, :], mq_d[0])
            nc.sync.dma_start(mq23[:, :], mq_d[1])
            for t in range(16):
                nc.sync.dma_start(mv2_sb[:, t, :], mv2_d[t])

            # qv projection, row-tiled pair: q01 = [q_h0; q_h1] from T0,
            # q23 = [q_h2; q_h3] from T8.
            for c in range(NI // 512):
                s = sc_psum.tile([128, 2, 512], F32, tag="sc")
                nc.tensor.matmul(s[:, 0, :], w_full[0:64, 0:128],
                                 f_full[0:64, ts(c, 512)],
                                 start=True, stop=True, tile_position=(0, 0))
                nc.tensor.matmul(s[:, 1, :], w_full[64:128, 128:256],
                                 f_full[64:128, ts(c, 512)],
                                 start=True, stop=True, tile_position=(64, 0))
                nc.vector.tensor_copy(q01[:, ts(c, 512)], s[:, 0, :])
                nc.vector.tensor_copy(q23[:, ts(c, 512)], s[:, 1, :])

            for p in range(2):
                he, ho = 2 * p, 2 * p + 1
                qsb = q01 if p == 0 else q23
                mqsb = mq01 if p == 0 else mq23

                # ---- prestage v^T for both heads of the pair ----
                ve = ve_pool.tile([128, IB, 2, 64], BF16, tag="ve")
                for ib in range(IB):
                    vt = sc_psum.tile([128, 2, 512], F32, tag="sc")
                    nc.tensor.matmul(vt[:, 0, 0:64], f_full[0:64, ts(ib, 128)],
                                     w_full[0:64, 256 + he * 64: 320 + he * 64],
                                     start=True, stop=True,
                                     tile_position=(0, 0))
                    nc.tensor.matmul(vt[:, 1, 0:64],
                                     f_full[64:128, ts(ib, 128)],
                                     w_full[64:128, 256 + ho * 64: 320 + ho * 64],
                                     start=True, stop=True,
                                     tile_position=(64, 0))
                    nc.vector.tensor_copy(ve[:, ib, :, :], vt[:, :, 0:64])

                # ---- pass 1: natural scores -> exp -> U + rowsums ----
                u_pair = acc_psum.tile([128, M], F32, tag="u")
                rs_e = sum_pool.tile([128, IB], F32, tag="rs_e")
                rs_o = sum_pool.tile([128, IB], F32, tag="rs_o")
                rs_sum = sum_pool.tile([128, IB], F32, tag="rs_sum")
                for g in range(IB):
                    s = sc_psum.tile([128, 2, 512], F32, tag="sc")
                    nc.tensor.matmul(s[:, 0, :], qsb[0:64, ts(g, 128)],
                                     mqsb[0:64, :], start=True, stop=True,
                                     tile_position=(0, 0))
                    nc.tensor.matmul(s[:, 1, :], qsb[64:128, ts(g, 128)],
                                     mqsb[64:128, :], start=True, stop=True,
                                     tile_position=(64, 0))
                    en = en_pool.tile([128, 2, 512], BF16, tag="en")
                    nc.scalar.activation(en[:, :, :], s[:, :, :], EXP,
                                         scale=SCALE,
                                         accum_out=rs_sum[:, g:g + 1])
                    nc.vector.tensor_reduce(rs_e[:, g:g + 1], en[:, 0, :],
                                            axis=AXX, op=ADD)
                    nc.tensor.matmul(u_pair[0:64, :], ve[:, g, 0, :],
                                     en[:, 0, :], start=(g == 0),
                                     stop=(g == IB - 1), tile_position=(0, 0),
                                     skip_group_check=True)
                    nc.tensor.matmul(u_pair[64:128, :], ve[:, g, 1, :],
                                     en[:, 1, :], start=(g == 0),
                                     stop=(g == IB - 1), tile_position=(0, 64),
                                     skip_group_check=True)
                nc.vector.tensor_sub(rs_o[:, :], rs_sum[:, :], rs_e[:, :])
                u_sb = out_pool.tile([128, M], F32, tag="u_sb")
                nc.vector.tensor_copy(u_sb[:, :], u_pair[:, :])
                nc.sync.dma_start(up_d[p], u_sb[:, :])
                nc.sync.dma_start(rs_d[he], rs_e[:, :])
                nc.sync.dma_start(rs_d[ho], rs_o[:, :])

                # ---- pass 2: transposed scores -> exp -> Pfeat + dens ----
                den_e = sum_pool.tile([128, 4 * NG2], F32, tag="den_e")
                den_o = sum_pool.tile([128, 4 * NG2], F32, tag="den_o")
                den_sum = sum_pool.tile([128, 4 * NG2], F32, tag="den_sum")
                for g in range(NG2):
                    pf_pair = acc_psum.tile([128, 512], F32, tag="pf")
                    for jb in range(4):
                        st = sc_psum.tile([128, 2, 512], F32, tag="sc")
                        nc.tensor.matmul(st[:, 0, :], mqsb[0:64, ts(jb, 128)],
                                         qsb[0:64, ts(g, 512)],
                                         start=True, stop=True,
                                         tile_position=(0, 0))
                        nc.tensor.matmul(st[:, 1, :],
                                         mqsb[64:128, ts(jb, 128)],
                                         qsb[64:128, ts(g, 512)],
                                         start=True, stop=True,
                                         tile_position=(64, 0))
                        et = et_pool.tile([128, 2, 512], BF16, tag="et")
                        col = jb * NG2 + g
                        nc.scalar.activation(et[:, :, :], st[:, :, :], EXP,
                                             scale=SCALE,
                                             accum_out=den_sum[:, col:col + 1])
                        nc.vector.tensor_reduce(den_e[:, col:col + 1],
                                                et[:, 0, :], axis=AXX, op=ADD)
                        nc.tensor.matmul(pf_pair[0:64, :],
                                         mv2_sb[:, he * 4 + jb, :],
                                         et[:, 0, :], start=(jb == 0),
                                         stop=(jb == 3), tile_position=(0, 0),
                                         skip_group_check=True)
                        nc.tensor.matmul(pf_pair[64:128, :],
                                         mv2_sb[:, ho * 4 + jb, :],
                                         et[:, 1, :], start=(jb == 0),
                                         stop=(jb == 3), tile_position=(0, 64),
                                         skip_group_check=True)
                    pf_sb = out_pool.tile([128, 512], F32, tag="pf_sb")
                    nc.vector.tensor_copy(pf_sb[:, :], pf_pair[:, :])
                    nc.sync.dma_start(pf_d[p, :, ts(g, 512)], pf_sb[:, :])
                nc.vector.tensor_sub(den_o[:, :], den_sum[:, :], den_e[:, :])
                nc.sync.dma_start(den_d[he], den_e[:, :])
                nc.sync.dma_start(den_d[ho], den_o[:, :])

    nc.compile()
    return nc


_PERM = np.array([d * 4 + h for h in range(H) for d in range(D)])


def host_prep(feat, semantic_map, w_feat_qv, w_map_qv, w_feat_out, NI):
    import ml_dtypes
    bf16 = ml_dtypes.bfloat16
    f = feat.reshape(B, C, N)
    m = semantic_map.reshape(B, 128, M)
    wq = w_feat_qv[:256][_PERM]
    wv = w_feat_qv[256:][_PERM]
    wqvT = np.ascontiguousarray(np.concatenate([wq, wv], 0).T)

    mqv = np.einsum('oc,bcm->bom', w_map_qv, m)
    map_q = mqv[:, :256][:, _PERM].reshape(B, H, D, M)
    map_v = mqv[:, 256:][:, _PERM].reshape(B, H, D, M)
    wfo_h = w_feat_out[:, _PERM].reshape(64, H, D).transpose(1, 0, 2)  # (H,64,D)

    mapq_dev = np.ascontiguousarray(map_q.reshape(B, 2, 2, D, M)
                                    .reshape(B, 2, 128, M))
    mv2 = np.empty((B, H, M, 64), np.float32)
    for b in range(B):
        for h in range(H):
            mv2[b, h] = (wfo_h[h] @ map_v[b, h]).T
    mv2_dev = np.ascontiguousarray(mv2.reshape(B, H, 4, 128, 64)
                                   .reshape(B, 16, 128, 64))

    in_maps = []
    for core in range(8):
        b, iq = core // 4, core % 4
        in_maps.append({
            "f": np.ascontiguousarray(f[b, :, iq * NI:(iq + 1) * NI]).astype(bf16),
            "wqvT": wqvT.astype(bf16),
            "mapq": mapq_dev[b].astype(bf16),
            "mv2e": mv2_dev[b].astype(bf16),
        })
    return in_maps


def host_post(results, w_map_out, NI):
    IB = NI // 128
    wmo_h = w_map_out[:, _PERM].reshape(128, H, D).transpose(1, 0, 2)  # (H,128,D)
    feat_out = np.empty((B, 64, N), np.float32)
    U = np.zeros((B, H, D, M), np.float32)
    den = np.zeros((B, H, M), np.float32)
    for core in range(8):
        b, iq = core // 4, core % 4
        r = results[core]
        acc = np.zeros((64, NI), np.float32)
        for h in range(H):
            pfh = r["pfp"][h // 2, (h % 2) * 64:(h % 2) * 64 + 64, :]  # (64,NI)
            rsh = r["rs"][h].T.reshape(NI)                   # [ib,128] -> i
            acc += pfh / rsh[None, :]
            U[b, h] += r["up"][h // 2, (h % 2) * 64:(h % 2) * 64 + 64, :]
            den[b, h] += r["den"][h].reshape(128, 4, -1).sum(2).T.reshape(M)
        feat_out[b, :, iq * NI:(iq + 1) * NI] = acc
    map_out = np.zeros((B, 128, M), np.float32)
    for b in range(B):
        for h in range(H):
            map_out[b] += wmo_h[h] @ (U[b, h] / den[b, h][None, :])
    return (feat_out.reshape(B, 64, 32, 32, 32),
            map_out.reshape(B, 128, 8, 8, 8))


_NC_CACHE = {}


def _get_nc(NI):
    if NI not in _NC_CACHE:
        _NC_CACHE[NI] = build_nc(NI)
    return _NC_CACHE[NI]


def kernel(feat, semantic_map, w_feat_qv, w_map_qv, w_feat_out, w_map_out,
           _trace=False):
    NI = N // 4
    feat = np.asarray(feat, np.float32)
    semantic_map = np.asarray(semantic_map, np.float32)
    w_feat_qv = np.asarray(w_feat_qv, np.float32)
    w_map_qv = np.asarray(w_map_qv, np.float32)
    w_feat_out = np.asarray(w_feat_out, np.float32)
    w_map_out = np.asarray(w_map_out, np.float32)

    nc = _get_nc(NI)
    in_maps = host_prep(feat, semantic_map, w_feat_qv, w_map_qv, w_feat_out, NI)
    res = run_bass_kernel_spmd(nc, in_maps, list(range(8)), trace=_trace)
    out = host_post(res.results, w_map_out, NI)
    if _trace:
        return out, res
    return out


# revision 15
# speedup vs baseline: 4.5300x; 1.0632x over previous
"""Bidirectional attention kernel for Trainium2 (8 NeuronCores, SPMD).

Sharding: core = (batch b, feat-token quarter iq). Each core processes
NI = 8192 feat tokens for one batch across all 4 heads.

Heads are processed in pairs (even head on SBUF partitions 0-63, odd head
on 64-127):
  - score matmuls (K=64) run as concurrent 64x128 row tiles T0/T8,
  - exp on ScalarE straight out of PSUM (bf16 out),
  - U (map-direction) and Pfeat (feat-direction) matmuls (K=128) run as
    concurrent 128x64 column tiles (even head -> PSUM 0-63, odd -> 64-127),
  - softmax row-sums (over map tokens) are free-dim reduces on E-natural,
    denominators (over feat tokens) are free-dim reduces on E-transposed,
    both on VectorE.
Host does the tiny map-side prep (map qv projection, MV2 = W_feat_out @
map_v precompute) and the final normalization / head-sum / map projection.
"""
import numpy as np

import concourse.bass as bass
import concourse.bacc as bacc
import concourse.mybir as mybir
import concourse.tile as tile
from concourse.bass import ts
from concourse.bass_utils import run_bass_kernel_spmd

F32 = mybir.dt.float32
BF16 = mybir.dt.bfloat16
EXP = mybir.ActivationFunctionType.Exp
AXX = mybir.AxisListType.X
ADD = mybir.AluOpType.add

B, C, H, D, M = 2, 64, 4, 64, 512
N = 32 * 32 * 32
SCALE = float(D) ** -0.5


def build_nc(NI=N // 4):
    IB = NI // 128            # 128-token i-blocks
    NG2 = NI // 512           # 512-token i-groups
    nc = bacc.Bacc("TRN2")

    f_d = nc.dram_tensor("f", [C, NI], BF16, kind="ExternalInput")
    w_d = nc.dram_tensor("wqvT", [C, 512], BF16, kind="ExternalInput")
    mq_d = nc.dram_tensor("mapq", [2, 128, M], BF16, kind="ExternalInput")
    mv2_d = nc.dram_tensor("mv2e", [16, 128, 64], BF16, kind="ExternalInput")
    pf_d = nc.dram_tensor("pfp", [2, 128, NI], F32, kind="ExternalOutput")
    up_d = nc.dram_tensor("up", [2, 128, M], F32, kind="ExternalOutput")
    rs_d = nc.dram_tensor("rs", [H, 128, IB], F32, kind="ExternalOutput")
    den_d = nc.dram_tensor("den", [H, 128, 4 * NG2], F32, kind="ExternalOutput")

    with tile.TileContext(nc) as tc:
        with (
            tc.tile_pool(name="const", bufs=1) as const_pool,
            tc.tile_pool(name="qp", bufs=1) as q_pool,
            tc.tile_pool(name="vep", bufs=2) as ve_pool,
            tc.tile_pool(name="enp", bufs=4) as en_pool,
            tc.tile_pool(name="etp", bufs=4) as et_pool,
            tc.tile_pool(name="sump", bufs=2) as sum_pool,
            tc.tile_pool(name="outp", bufs=3) as out_pool,
            tc.tile_pool(name="ps_sc", bufs=3, space="PSUM") as sc_psum,
            tc.tile_pool(name="ps_acc", bufs=1, space="PSUM") as acc_psum,
        ):
            f_full = const_pool.tile([128, NI], BF16)
            w_full = const_pool.tile([128, 512], BF16)
            mq01 = const_pool.tile([128, M], BF16)
            mq23 = const_pool.tile([128, M], BF16)
            mv2_sb = const_pool.tile([128, 16, 64], BF16)
            q01 = q_pool.tile([128, NI], BF16)
            q23 = q_pool.tile([128, NI], BF16)

            nc.sync.dma_start(f_full[0:64, :], f_d[:, :])
            nc.sync.dma_start(f_full[64:128, :], f_d[:, :])
            nc.sync.dma_start(w_full[0:64, :], w_d[:, :])
            nc.sync.dma_start(w_full[64:128, :], w_d[:, :])
            nc.sync.dma_start(mq01[:, :], mq_d[0])
            nc.sync.dma_start(mq23[:, :], mq_d[1])
            for t in range(16):
                nc.sync.dma_start(mv2_sb[:, t, :], mv2_d[t])

            # qv projection, row-tiled pair: q01 = [q_h0; q_h1] from T0,
            # q23 = [q_h2; q_h3] from T8.
            for c in range(NI // 512):
                s = sc_psum.tile([128, 2, 512], F32, tag="sc")
                nc.tensor.matmul(s[:, 0, :], w_full[0:64, 0:128],
                                 f_full[0:64, ts(c, 512)],
                                 start=True, stop=True, tile_position=(0, 0))
                nc.tensor.matmul(s[:, 1, :], w_full[64:128, 128:256],
                                 f_full[64:128, ts(c, 512)],
                                 start=True, stop=True, tile_position=(64, 0))
                nc.vector.tensor_copy(q01[:, ts(c, 512)], s[:, 0, :])
                nc.vector.tensor_copy(q23[:, ts(c, 512)], s[:, 1, :])

            for p in range(2):
                he, ho = 2 * p, 2 * p + 1
                qsb = q01 if p == 0 else q23
                mqsb = mq01 if p == 0 else mq23

                # ---- v^T staging is interleaved into pass 1 (+4 blocks
                # lookahead) so ScalarE starts exp work immediately ----
                ve = ve_pool.tile([128, IB, 2, 64], BF16, tag="ve")

                def stage_vt(ib):
                    vt = sc_psum.tile([128, 2, 512], F32, tag="sc")
                    nc.tensor.matmul(vt[:, 0, 0:64], f_full[0:64, ts(ib, 128)],
                                     w_full[0:64, 256 + he * 64: 320 + he * 64],
                                     start=True, stop=True,
                                     tile_position=(0, 0))
                    nc.tensor.matmul(vt[:, 1, 0:64],
                                     f_full[64:128, ts(ib, 128)],
                                     w_full[64:128, 256 + ho * 64: 320 + ho * 64],
                                     start=True, stop=True,
                                     tile_position=(64, 0))
                    nc.vector.tensor_copy(ve[:, ib, :, :], vt[:, :, 0:64])

                for ib in range(4):
                    stage_vt(ib)

                # ---- pass 1: natural scores -> exp -> U + rowsums ----
                u_pair = acc_psum.tile([128, M], F32, tag="u")
                rs_e = sum_pool.tile([128, IB], F32, tag="rs_e")
                rs_o = sum_pool.tile([128, IB], F32, tag="rs_o")
                rs_sum = sum_pool.tile([128, IB], F32, tag="rs_sum")
                for g in range(IB):
                    s = sc_psum.tile([128, 2, 512], F32, tag="sc")
                    nc.tensor.matmul(s[:, 0, :], qsb[0:64, ts(g, 128)],
                                     mqsb[0:64, :], start=True, stop=True,
                                     tile_position=(0, 0))
                    nc.tensor.matmul(s[:, 1, :], qsb[64:128, ts(g, 128)],
                                     mqsb[64:128, :], start=True, stop=True,
                                     tile_position=(64, 0))
                    en = en_pool.tile([128, 2, 512], BF16, tag="en")
                    nc.scalar.activation(en[:, :, :], s[:, :, :], EXP,
                                         scale=SCALE,
                                         accum_out=rs_sum[:, g:g + 1])
                    nc.vector.tensor_reduce(rs_e[:, g:g + 1], en[:, 0, :],
                                            axis=AXX, op=ADD)
                    nc.tensor.matmul(u_pair[0:64, :], ve[:, g, 0, :],
                                     en[:, 0, :], start=(g == 0),
                                     stop=(g == IB - 1), tile_position=(0, 0),
                                     skip_group_check=True)
                    nc.tensor.matmul(u_pair[64:128, :], ve[:, g, 1, :],
                                     en[:, 1, :], start=(g == 0),
                                     stop=(g == IB - 1), tile_position=(0, 64),
                                     skip_group_check=True)
                    if g + 4 < IB:
                        stage_vt(g + 4)
                nc.vector.tensor_sub(rs_o[:, :], rs_sum[:, :], rs_e[:, :])
                u_sb = out_pool.tile([128, M], F32, tag="u_sb")
                nc.vector.tensor_copy(u_sb[:, :], u_pair[:, :])
                nc.sync.dma_start(up_d[p], u_sb[:, :])
                nc.sync.dma_start(rs_d[he], rs_e[:, :])
                nc.sync.dma_start(rs_d[ho], rs_o[:, :])

                # ---- pass 2: transposed scores -> exp -> Pfeat + dens ----
                den_e = sum_pool.tile([128, 4 * NG2], F32, tag="den_e")
                den_o = sum_pool.tile([128, 4 * NG2], F32, tag="den_o")
                den_sum = sum_pool.tile([128, 4 * NG2], F32, tag="den_sum")
                for g in range(NG2):
                    pf_pair = acc_psum.tile([128, 512], F32, tag="pf")
                    for jb in range(4):
                        st = sc_psum.tile([128, 2, 512], F32, tag="sc")
                        nc.tensor.matmul(st[:, 0, :], mqsb[0:64, ts(jb, 128)],
                                         qsb[0:64, ts(g, 512)],
                                         start=True, stop=True,
                                         tile_position=(0, 0))
                        nc.tensor.matmul(st[:, 1, :],
                                         mqsb[64:128, ts(jb, 128)],
                                         qsb[64:128, ts(g, 512)],
                                         start=True, stop=True,
                                         tile_position=(64, 0))
                        et = et_pool.tile([128, 2, 512], BF16, tag="et")
                        col = jb * NG2 + g
                        nc.scalar.activation(et[:, :, :], st[:, :, :], EXP,
                                             scale=SCALE,
                                             accum_out=den_sum[:, col:col + 1])
                        nc.vector.tensor_reduce(den_e[:, col:col + 1],
                                                et[:, 0, :], axis=AXX, op=ADD)
                        nc.tensor.matmul(pf_pair[0:64, :],
                                         mv2_sb[:, he * 4 + jb, :],
                                         et[:, 0, :], start=(jb == 0),
                                         stop=(jb == 3), tile_position=(0, 0),
                                         skip_group_check=True)
                        nc.tensor.matmul(pf_pair[64:128, :],
                                         mv2_sb[:, ho * 4 + jb, :],
                                         et[:, 1, :], start=(jb == 0),
                                         stop=(jb == 3), tile_position=(0, 64),
                                         skip_group_check=True)
                    pf_sb = out_pool.tile([128, 512], F32, tag="pf_sb")
                    nc.vector.tensor_copy(pf_sb[:, :], pf_pair[:, :])
                    nc.sync.dma_start(pf_d[p, :, ts(g, 512)], pf_sb[:, :])
                nc.vector.tensor_sub(den_o[:, :], den_sum[:, :], den_e[:, :])
                nc.sync.dma_start(den_d[he], den_e[:, :])
                nc.sync.dma_start(den_d[ho], den_o[:, :])

    nc.compile()
    return nc


_PERM = np.array([d * 4 + h for h in range(H) for d in range(D)])


def host_prep(feat, semantic_map, w_feat_qv, w_map_qv, w_feat_out, NI):
    import ml_dtypes
    bf16 = ml_dtypes.bfloat16
    f = feat.reshape(B, C, N)
    m = semantic_map.reshape(B, 128, M)
    wq = w_feat_qv[:256][_PERM]
    wv = w_feat_qv[256:][_PERM]
    wqvT = np.ascontiguousarray(np.concatenate([wq, wv], 0).T)

    mqv = np.einsum('oc,bcm->bom', w_map_qv, m)
    map_q = mqv[:, :256][:, _PERM].reshape(B, H, D, M)
    map_v = mqv[:, 256:][:, _PERM].reshape(B, H, D, M)
    wfo_h = w_feat_out[:, _PERM].reshape(64, H, D).transpose(1, 0, 2)  # (H,64,D)

    mapq_dev = np.ascontiguousarray(map_q.reshape(B, 2, 2, D, M)
                                    .reshape(B, 2, 128, M))
    mv2 = np.empty((B, H, M, 64), np.float32)
    for b in range(B):
        for h in range(H):
            mv2[b, h] = (wfo_h[h] @ map_v[b, h]).T
    mv2_dev = np.ascontiguousarray(mv2.reshape(B, H, 4, 128, 64)
                                   .reshape(B, 16, 128, 64))

    in_maps = []
    for core in range(8):
        b, iq = core // 4, core % 4
        in_maps.append({
            "f": np.ascontiguousarray(f[b, :, iq * NI:(iq + 1) * NI]).astype(bf16),
            "wqvT": wqvT.astype(bf16),
            "mapq": mapq_dev[b].astype(bf16),
            "mv2e": mv2_dev[b].astype(bf16),
        })
    return in_maps


def host_post(results, w_map_out, NI):
    IB = NI // 128
    wmo_h = w_map_out[:, _PERM].reshape(128, H, D).transpose(1, 0, 2)  # (H,128,D)
    feat_out = np.empty((B, 64, N), np.float32)
    U = np.zeros((B, H, D, M), np.float32)
    den = np.zeros((B, H, M), np.float32)
    for core in range(8):
        b, iq = core // 4, core % 4
        r = results[core]
        acc = np.zeros((64, NI), np.float32)
        for h in range(H):
            pfh = r["pfp"][h // 2, (h % 2) * 64:(h % 2) * 64 + 64, :]  # (64,NI)
            rsh = r["rs"][h].T.reshape(NI)                   # [ib,128] -> i
            acc += pfh / rsh[None, :]
            U[b, h] += r["up"][h // 2, (h % 2) * 64:(h % 2) * 64 + 64, :]
            den[b, h] += r["den"][h].reshape(128, 4, -1).sum(2).T.reshape(M)
        feat_out[b, :, iq * NI:(iq + 1) * NI] = acc
    map_out = np.zeros((B, 128, M), np.float32)
    for b in range(B):
        for h in range(H):
            map_out[b] += wmo_h[h] @ (U[b, h] / den[b, h][None, :])
    return (feat_out.reshape(B, 64, 32, 32, 32),
            map_out.reshape(B, 128, 8, 8, 8))


_NC_CACHE = {}


def _get_nc(NI):
    if NI not in _NC_CACHE:
        _NC_CACHE[NI] = build_nc(NI)
    return _NC_CACHE[NI]


def kernel(feat, semantic_map, w_feat_qv, w_map_qv, w_feat_out, w_map_out,
           _trace=False):
    NI = N // 4
    feat = np.asarray(feat, np.float32)
    semantic_map = np.asarray(semantic_map, np.float32)
    w_feat_qv = np.asarray(w_feat_qv, np.float32)
    w_map_qv = np.asarray(w_map_qv, np.float32)
    w_feat_out = np.asarray(w_feat_out, np.float32)
    w_map_out = np.asarray(w_map_out, np.float32)

    nc = _get_nc(NI)
    in_maps = host_prep(feat, semantic_map, w_feat_qv, w_map_qv, w_feat_out, NI)
    res = run_bass_kernel_spmd(nc, in_maps, list(range(8)), trace=_trace)
    out = host_post(res.results, w_map_out, NI)
    if _trace:
        return out, res
    return out
